# revision 8
# baseline (speedup 1.0000x reference)
"""Trainium2 Bass kernel for nn_IterativeDecimator (GNN coarsening).

Computes, given node features X [N,D], a tiny MLP (D->32->K) + softmax
assignment matrix S [N,K], then:
  coarse_nodes = S^T X                     [K, D]
  coarse_adj   = S^T A S                   [K, K]   (A = dedup'd edge adjacency)
  c_senders/c_receivers/c_edges = nonzero extraction of coarse_adj
  assignments  = S

Strategy (8 NeuronCores, SPMD):
  - Nodes row-sharded 2048/core. Each core runs the MLP+softmax on its
    shard (fp32, tensor engine), computes the S^T X partial, and
    contributes its S shard to an AllGather.
  - Edges are dedup'd host-side (adjacency .set(1.0) semantics), padded
    with (0,0) self-edges to 65536/core (the pad contribution is
    subtracted exactly on host), and row-gathered out of the all-gathered
    S table with dma_gather (256B rows). coarse_adj partial accumulates
    in PSUM over 512 [128-edge] matmuls per core.
  - One fused AllReduce reduces the [K, D+K] (coarse_nodes | coarse_adj)
    partials; nonzero extraction of the [64,64] result is a host epilogue.
"""

import sys

for _p in ("/opt/trn_rl_repo",):
    if _p not in sys.path:
        sys.path.insert(0, _p)

import numpy as np

import concourse.bass as bass
import concourse.mybir as mybir
import concourse.bacc as bacc
import concourse.tile as tile
import concourse.bass_utils as bass_utils
from concourse.masks import make_identity

FP = mybir.dt.float32

# Full-problem config (hardcoded per contract).
CFG = dict(
    N=16384,   # nodes
    D=256,     # latent
    H=32,      # hidden
    K=64,      # clusters
    NC=8,      # cores
    EC=65536,  # padded edges per core
    GCH=8192,  # edge-gather chunk (indices per dma_gather)
)


def build_kernel(tc, outs, ins, cfg):
    """Trace the per-core SPMD program into TileContext `tc`.

    ins/outs: dicts of DRAM APs:
      ins:  xt [D, Nc], xn [Nc, D], w1 [D, H], b1 [H], w2 [H, K], b2 [K],
            sidx [128, EC//16] i16, ridx [128, EC//16] i16
      outs: s_out [Nc, K], cn_out [K, D], ca_out [K, K]
    """
    nc = tc.nc
    N, D, H, K, NC = cfg["N"], cfg["D"], cfg["H"], cfg["K"], cfg["NC"]
    EC, GCH = cfg["EC"], cfg["GCH"]
    Nc = N // NC          # nodes per core
    NT = Nc // 128        # 128-node tiles per core
    CT = Nc // 512 if Nc >= 512 else 1   # MLP column tiles
    CW = min(Nc, 512)     # MLP column tile width
    DT = D // 128         # contraction chunks for X@W1
    NCHUNK = EC // GCH    # gather chunks
    SLOTS = GCH // 128    # 128-edge matmul slots per chunk

    with (
        tc.tile_pool(name="const", bufs=1) as constp,
        tc.tile_pool(name="acts", bufs=1) as acts,
        tc.tile_pool(name="mlp", bufs=2) as mlp,
        tc.tile_pool(name="stat", bufs=2) as stat,
        tc.tile_pool(name="gat", bufs=2) as gat,
        tc.tile_pool(name="ph", bufs=1, space="PSUM") as ph,
        tc.tile_pool(name="pl", bufs=1, space="PSUM") as pl,
        tc.tile_pool(name="pt", bufs=2, space="PSUM") as pt,
        tc.tile_pool(name="pacc", bufs=2, space="PSUM") as pacc,
        tc.tile_pool(name="dram", bufs=1, space="DRAM") as dram,
    ):
        # ---- constants / inputs to SBUF ----
        w1_sb = constp.tile([128, DT, H], FP)
        nc.sync.dma_start(w1_sb[:], ins["w1"].rearrange("(c p) h -> p c h", p=128))
        w2_sb = constp.tile([H, K], FP)
        nc.sync.dma_start(w2_sb[:], ins["w2"][:])
        b1_sb = constp.tile([H, 1], FP)
        nc.sync.dma_start(b1_sb[:], ins["b1"][:, None])
        b2_sb = constp.tile([K, 1], FP)
        nc.sync.dma_start(b2_sb[:], ins["b2"][:, None])
        ident = constp.tile([K, K], FP)
        make_identity(nc, ident[:])

        xt_sb = acts.tile([128, DT, Nc], FP)
        nc.sync.dma_start(xt_sb[:], ins["xt"].rearrange("(c p) n -> p c n", p=128))
        xn_sb = acts.tile([128, NT, D], FP)
        nc.sync.dma_start(xn_sb[:], ins["xn"].rearrange("(t p) d -> p t d", p=128))

        sidx_sb = acts.tile([128, EC // 128], mybir.dt.int32)
        nc.sync.dma_start(sidx_sb[:], ins["sidx"][:])
        ridx_sb = acts.tile([128, EC // 128], mybir.dt.int32)
        nc.sync.dma_start(ridx_sb[:], ins["ridx"][:])

        # ---- MLP + softmax -> S (node-major, [128, NT, K] slots) ----
        s_all = acts.tile([128, NT, K], FP)
        for t in range(CT):
            cs = t * CW
            psum_h = ph.tile([H, CW], FP, space="PSUM")
            for cchunk in range(DT):
                nc.tensor.matmul(
                    psum_h[:],
                    w1_sb[:, cchunk, :],
                    xt_sb[:, cchunk, cs:cs + CW],
                    start=(cchunk == 0),
                    stop=(cchunk == DT - 1),
                )
            h_sb = mlp.tile([H, CW], FP)
            nc.scalar.activation(
                h_sb[:], psum_h[:], mybir.ActivationFunctionType.Relu, bias=b1_sb[:]
            )
            psum_l = pl.tile([K, CW], FP, space="PSUM")
            nc.tensor.matmul(psum_l[:], w2_sb[:], h_sb[:], start=True, stop=True)
            lg_sb = mlp.tile([K, CW], FP)
            nc.vector.tensor_scalar_add(lg_sb[:], psum_l[:], b2_sb[:])
            for j in range(CW // 128):
                ti = (cs + j * 128) // 128
                psum_t = pt.tile([128, K], FP, space="PSUM")
                nc.tensor.transpose(
                    psum_t[:], lg_sb[:, j * 128:(j + 1) * 128], ident[:]
                )
                s_slot = s_all[:, ti, :]
                mx = stat.tile([128, 1], FP)
                nc.vector.reduce_max(mx[:], psum_t[:], axis=mybir.AxisListType.X)
                nm = stat.tile([128, 1], FP)
                nc.scalar.mul(nm[:], mx[:], -1.0)
                nc.scalar.activation(
                    s_slot, psum_t[:], mybir.ActivationFunctionType.Exp, bias=nm[:]
                )
                sm = stat.tile([128, 1], FP)
                nc.vector.reduce_sum(sm[:], s_slot, axis=mybir.AxisListType.X)
                rv = stat.tile([128, 1], FP)
                nc.vector.reciprocal(rv[:], sm[:])
                nc.vector.tensor_scalar_mul(s_slot, s_slot, rv[:])

        # ---- S shard out + AllGather full S table ----
        s_dram_view = outs["s_out"].rearrange("(t p) k -> p t k", p=128)
        nc.sync.dma_start(s_dram_view, s_all[:])
        ag_in = dram.tile([Nc, K], FP)
        nc.sync.dma_start(
            ag_in.opt().rearrange("(t p) k -> p t k", p=128), s_all[:]
        )
        ag_out = dram.tile([N, K], FP)
        if cfg.get("no_coll"):
            for c in range(NC):
                nc.sync.dma_start(
                    ag_out[c * Nc:(c + 1) * Nc].rearrange(
                        "(t p) k -> p t k", p=128), s_all[:])
        else:
            nc.gpsimd.collective_compute(
                "AllGather",
                mybir.AluOpType.bypass,
                replica_groups=[list(range(NC))],
                ins=[ag_in.opt()],
                outs=[ag_out.opt()],
            )

        # ---- coarse_nodes partial: S_c^T @ X_c -> psum_cn [K, D] ----
        psum_cn = pacc.tile([K, D], FP, space="PSUM", tag="pcn")
        for t in range(NT):
            nc.tensor.matmul(
                psum_cn[:],
                s_all[:, t, :],
                xn_sb[:, t, :],
                start=(t == 0),
                stop=(t == NT - 1),
            )
        ar_sb = acts.tile([K, D + K], FP)
        nc.vector.tensor_copy(ar_sb[:, :D], psum_cn[:])

        # ---- edge gathers + coarse_adj partial ----
        psum_ca = pacc.tile([K, K], FP, space="PSUM", tag="pca")
        icols = GCH // 16
        for c in range(NCHUNK):
            gs = gat.tile([128, SLOTS, K], FP, tag="gs")
            gr = gat.tile([128, SLOTS, K], FP, tag="gr")
            if cfg.get("no_gather"):
                src = ag_out[(c * GCH) % N:((c * GCH) % N) + GCH].rearrange(
                    "(s p) k -> p s k", p=128)
                nc.sync.dma_start(gs[:], src)
                nc.sync.dma_start(gr[:], src)
            else:
                for s in range(SLOTS):
                    col = c * SLOTS + s
                    nc.gpsimd.indirect_dma_start(
                        out=gs[:, s, :], out_offset=None,
                        in_=ag_out.opt(),
                        in_offset=bass.IndirectOffsetOnAxis(
                            ap=sidx_sb[:, col:col + 1], axis=0),
                    )
                    nc.gpsimd.indirect_dma_start(
                        out=gr[:, s, :], out_offset=None,
                        in_=ag_out.opt(),
                        in_offset=bass.IndirectOffsetOnAxis(
                            ap=ridx_sb[:, col:col + 1], axis=0),
                    )
            for s in range(SLOTS):
                nc.tensor.matmul(
                    psum_ca[:],
                    gs[:, s, :],
                    gr[:, s, :],
                    start=(c == 0 and s == 0),
                    stop=(c == NCHUNK - 1 and s == SLOTS - 1),
                )
        nc.vector.tensor_copy(ar_sb[:, D:], psum_ca[:])

        # ---- fused AllReduce of [K, D+K] partials ----
        ar_in = dram.tile([K, D + K], FP)
        nc.sync.dma_start(ar_in.opt(), ar_sb[:])
        ar_out = dram.tile([K, D + K], FP)
        if cfg.get("no_coll"):
            nc.sync.dma_start(ar_out.opt(), ar_in.opt())
        else:
            nc.gpsimd.collective_compute(
                "AllReduce",
                mybir.AluOpType.add,
                replica_groups=[list(range(NC))],
                ins=[ar_in.opt()],
                outs=[ar_out.opt()],
            )
        red_sb = acts.tile([K, D + K], FP)
        nc.sync.dma_start(red_sb[:], ar_out.opt())
        nc.sync.dma_start(outs["cn_out"][:], red_sb[:, :D])
        nc.sync.dma_start(outs["ca_out"][:], red_sb[:, D:])


def build_program(cfg):
    """Build + compile the Bacc program. Returns (nc, names)."""
    N, D, H, K, NC = cfg["N"], cfg["D"], cfg["H"], cfg["K"], cfg["NC"]
    EC = cfg["EC"]
    Nc = N // NC
    nc = bacc.Bacc("TRN2", target_bir_lowering=False, debug=False,
                   enable_asserts=False, num_devices=NC)
    ins = {
        "xt": nc.dram_tensor("xt", [D, Nc], FP, kind="ExternalInput").ap(),
        "xn": nc.dram_tensor("xn", [Nc, D], FP, kind="ExternalInput").ap(),
        "w1": nc.dram_tensor("w1", [D, H], FP, kind="ExternalInput").ap(),
        "b1": nc.dram_tensor("b1", [H], FP, kind="ExternalInput").ap(),
        "w2": nc.dram_tensor("w2", [H, K], FP, kind="ExternalInput").ap(),
        "b2": nc.dram_tensor("b2", [K], FP, kind="ExternalInput").ap(),
        "sidx": nc.dram_tensor("sidx", [128, EC // 128], mybir.dt.int32,
                               kind="ExternalInput").ap(),
        "ridx": nc.dram_tensor("ridx", [128, EC // 128], mybir.dt.int32,
                               kind="ExternalInput").ap(),
    }
    outs = {
        "s_out": nc.dram_tensor("s_out", [Nc, K], FP, kind="ExternalOutput").ap(),
        "cn_out": nc.dram_tensor("cn_out", [K, D], FP, kind="ExternalOutput").ap(),
        "ca_out": nc.dram_tensor("ca_out", [K, K], FP, kind="ExternalOutput").ap(),
    }
    with tile.TileContext(nc) as tc:
        build_kernel(tc, outs, ins, cfg)
    nc.compile()
    return nc


def make_in_maps(node_feats, W1, b1, W2, b2, senders, receivers, cfg):
    """Host-side sharding/preprocessing. Returns (in_maps, pad_count)."""
    N, NC, EC = cfg["N"], cfg["NC"], cfg["EC"]
    Nc = N // NC
    X = np.ascontiguousarray(node_feats, dtype=np.float32)
    keys = senders.astype(np.int64) * N + receivers.astype(np.int64)
    uniq = np.unique(keys)
    E_u = uniq.shape[0]
    E_pad = NC * EC
    assert E_u <= E_pad
    pad_count = E_pad - E_u
    s_u = np.empty(E_pad, np.int64)
    r_u = np.empty(E_pad, np.int64)
    s_u[:E_u] = uniq // N
    r_u[:E_u] = uniq % N
    s_u[E_u:] = 0
    r_u[E_u:] = 0

    def wrap_idx(v):
        # slot s, partition p -> edge s*128 + p  (one index per partition)
        return np.ascontiguousarray(v.astype(np.int32).reshape(-1, 128).T)

    in_maps = []
    for c in range(NC):
        xs = X[c * Nc:(c + 1) * Nc]
        sl = slice(c * EC, (c + 1) * EC)
        in_maps.append({
            "xt": np.ascontiguousarray(xs.T),
            "xn": xs,
            "w1": np.ascontiguousarray(W1, np.float32),
            "b1": np.ascontiguousarray(b1, np.float32),
            "w2": np.ascontiguousarray(W2, np.float32),
            "b2": np.ascontiguousarray(b2, np.float32),
            "sidx": wrap_idx(s_u[sl]),
            "ridx": wrap_idx(r_u[sl]),
        })
    return in_maps, pad_count


_PROG = None


def kernel(node_feats, W1, b1, W2, b2, senders, receivers, **_ignored):
    global _PROG
    cfg = CFG
    if _PROG is None:
        _PROG = build_program(cfg)
    nc = _PROG
    in_maps, pad_count = make_in_maps(
        node_feats, W1, b1, W2, b2, senders, receivers, cfg)
    res = bass_utils.run_bass_kernel_spmd(
        nc, in_maps, core_ids=list(range(cfg["NC"])))
    results = res.results
    N, K = cfg["N"], cfg["K"]
    S = np.concatenate([results[c]["s_out"] for c in range(cfg["NC"])], axis=0)
    coarse_nodes = results[0]["cn_out"]
    CA = results[0]["ca_out"].copy()
    # remove the host-padding (0,0) self-edge contributions exactly
    CA -= float(pad_count) * np.outer(S[0], S[0]).astype(np.float32)
    flat = CA.reshape(-1)
    nz = np.nonzero(flat)[0]
    n = min(nz.shape[0], K * K)
    cs = np.zeros(K * K, np.int32)
    cr = np.zeros(K * K, np.int32)
    cs[:n] = (nz[:n] // K).astype(np.int32)
    cr[:n] = (nz[:n] % K).astype(np.int32)
    c_edges = CA[cs, cr][:, None].astype(np.float32)
    return (coarse_nodes, cs, cr, c_edges, S)
